# revision 4
# baseline (speedup 1.0000x reference)
"""Bahdanau-attention kernel for Trainium2 (8 NeuronCores, data-parallel over batch).

Computation (per batch b):
    enc_proj = h_enc @ W1.T + b1          # (L, D)   -- the big matmul
    dec_proj = h_dec @ W2.T + b2          # (D,)
    h        = tanh(enc_proj + dec_proj)  # (L, D)
    scores   = h @ V (+ bv)               # (L,)  -- bv cancels in softmax, dropped
    attn     = softmax(scores)            # no-max softmax: scores ~ N(0,0.4), exp safe
    ctx      = attn @ enc_proj            # (D,)

v2 design vs v1:
  - h_enc is shipped host-side pre-transposed + fp16 ("T-space": d on partitions),
    eliminating the on-device cast-DMA and xbar transpose entirely.
  - dec_proj + b1 + b2 computed on host (bias prep, 0.003% of FLOPs).
  - ctx via associativity: ctx = (attn @ h_enc) @ W1.T + b1, so enc_proj is never
    materialized to SBUF (no ACT evacuation). r = exp(scores) @ h_enc runs on DVE
    as fused tensor_tensor_reduce; the tiny (r/Z) @ W1.T matmul runs once per
    core with all 4 batches packed into M=4.
  - PE stream per half: 8 x (8 chained matmuls N=512) for enc_projT + 16 scores
    matmuls (replicated-V trick), software-pipelined so scores(c-1) issues
    between main(c) groups and never head-blocks the PE queue.
"""

import numpy as np

B, L, D = 32, 2048, 1024
NCORES = 8
NB = B // NCORES  # batches per core
P = 128
NCH = D // P      # 8 chunks of the d/e dimension
NH = 2            # l-halves per batch
LH = L // NH      # 1024

_cache = {}


def _build():
    import concourse.tile as tile
    from concourse import bacc, mybir
    from concourse.bass import ts, ds
    from contextlib import ExitStack

    FP16 = mybir.dt.float16
    FP32 = mybir.dt.float32
    Alu = mybir.AluOpType
    Act = mybir.ActivationFunctionType
    X = mybir.AxisListType.X

    nc = bacc.Bacc("TRN2", name="bahdanau_attn_v2")

    # hq[b, q, c, l] = fp16(h_enc[b, l, c*128+q])  (host pre-transposed)
    hq_d = nc.dram_tensor("hq", [NB, P, NCH, L], FP16, kind="ExternalInput")
    w1t = nc.dram_tensor("w1t", [NCH, P, D], FP16, kind="ExternalInput")  # [dchunk, dpart, e]
    vrep_d = nc.dram_tensor("vrep", [P, NCH, P], FP16, kind="ExternalInput")
    bias_d = nc.dram_tensor("biasd", [P, NCH, NB], FP32, kind="ExternalInput")
    b1r_d = nc.dram_tensor("b1r", [NB, D], FP32, kind="ExternalInput")
    out = nc.dram_tensor("ctx_out", [NB, D], FP32, kind="ExternalOutput")

    with tile.TileContext(nc) as tc, ExitStack() as ctx:
        wp = ctx.enter_context(tc.tile_pool(name="weights", bufs=1))
        ld = ctx.enter_context(tc.tile_pool(name="loads", bufs=3))
        hp = ctx.enter_context(tc.tile_pool(name="htan", bufs=4))
        xp = ctx.enter_context(tc.tile_pool(name="exps", bufs=2))
        sp = ctx.enter_context(tc.tile_pool(name="scratch", bufs=2))
        fin = ctx.enter_context(tc.tile_pool(name="final", bufs=2))
        psA = ctx.enter_context(tc.tile_pool(name="psA", bufs=3, space="PSUM"))
        psS = ctx.enter_context(tc.tile_pool(name="psS", bufs=1, space="PSUM"))
        psF = ctx.enter_context(tc.tile_pool(name="psF", bufs=1, space="PSUM"))

        # ---- prologue: weights / constants ----
        w1_sb = [wp.tile([P, D], FP16, tag=f"w1_{d}", name=f"w1_{d}") for d in range(NCH)]
        for d in range(NCH):
            nc.scalar.dma_start(w1_sb[d], w1t[d])
        vrep = wp.tile([P, NCH, P], FP16)
        nc.scalar.dma_start(vrep, vrep_d[:])
        bias_sb = wp.tile([P, NCH, NB], FP32)
        nc.scalar.dma_start(bias_sb, bias_d[:])
        b1r_sb = wp.tile([NB, D], FP32)
        nc.scalar.dma_start(b1r_sb, b1r_d[:])

        # r16_all[:, dc, b] = fp16((exp@h_enc / Z)[b, dc*128+q]) -- persists all batches
        r16_all = wp.tile([P, NCH, NB], FP16, tag="r16")

        # ---- main loop over batches ----
        for b in range(NB):
            exp_rep = xp.tile([P, L], FP16, tag="exp")  # exp(scores), replicated rows
            zsl = fin.tile([P, NH], FP32, tag="zsl")
            r_sl = fin.tile([P, NCH, NH], FP32, tag="rsl")

            for h in range(NH):
                hq_t = ld.tile([P, NCH, LH], FP16, tag="hq")
                nc.sync.dma_start(hq_t, hq_d[b, :, :, ds(h * LH, LH)])

                ps_sc = psS.tile([P, LH], FP32, tag="sc")
                prev = None  # software pipeline: scores(c-1) issue between main(c)
                for c in range(NCH):
                    hts = []
                    for g in range(2):
                        ps = psA.tile([P, 512], FP32, tag="mm")
                        for dc in range(NCH):
                            nc.tensor.matmul(
                                ps,
                                lhsT=w1_sb[dc][:, ts(c, P)],
                                rhs=hq_t[:, dc, ds(g * 512, 512)],
                                start=(dc == 0),
                                stop=(dc == NCH - 1),
                            )
                        # tanh(enc_projT + dec_proj + b1 + b2), fused bias on ACT
                        ht = hp.tile([P, 512], FP16, tag="ht")
                        nc.scalar.activation(ht, ps, Act.Tanh, bias=bias_sb[:, c, b : b + 1])
                        hts.append(ht)
                    if prev is not None:
                        pc, phts = prev
                        for g in range(2):
                            nc.tensor.matmul(
                                ps_sc[:, ts(g, 512)], lhsT=vrep[:, pc, :], rhs=phts[g],
                                start=(pc == 0), stop=False,
                            )
                    prev = (c, hts)
                pc, phts = prev
                for g in range(2):
                    nc.tensor.matmul(
                        ps_sc[:, ts(g, 512)], lhsT=vrep[:, pc, :], rhs=phts[g],
                        start=False, stop=True,
                    )
                # exp(scores) + per-partition Z, fused on ACT
                nc.scalar.activation(
                    exp_rep[:, ds(h * LH, LH)], ps_sc, Act.Exp,
                    accum_out=zsl[:, h : h + 1],
                )
                # r partials on DVE: r[q,c] += sum_l exp[l] * hq[q,c,l]
                with nc.allow_low_precision("fp16 product scratch; |e*h| < 40"):
                    for c in range(NCH):
                        scr = sp.tile([P, LH], FP16, tag="scr")
                        nc.vector.tensor_tensor_reduce(
                            out=scr,
                            in0=hq_t[:, c, :],
                            in1=exp_rep[:, ds(h * LH, LH)],
                            scale=1.0,
                            scalar=0.0,
                            op0=Alu.mult,
                            op1=Alu.add,
                            accum_out=r_sl[:, c, h : h + 1],
                        )

            # finalize batch: r16 = (r_h0 + r_h1) / Z
            zsum = fin.tile([P, 1], FP32, tag="zsum")
            nc.vector.tensor_reduce(zsum, zsl, axis=X, op=Alu.add)
            recip = fin.tile([P, 1], FP32, tag="recip")
            nc.vector.reciprocal(recip, zsum)
            rsum = fin.tile([P, NCH], FP32, tag="rsum")
            nc.vector.tensor_reduce(rsum, r_sl, axis=X, op=Alu.add)
            nc.vector.tensor_scalar(
                out=r16_all[:, :, b], in0=rsum, scalar1=recip, scalar2=None, op0=Alu.mult
            )

        # ---- core end: ctx = r16_all.T @ W1.T + b1, all batches packed M=4 ----
        psf = psF.tile([NB, D], FP32, tag="f")
        for w in range(2):
            for dc in range(NCH):
                nc.tensor.matmul(
                    psf[:, ts(w, 512)],
                    lhsT=r16_all[:, dc, :],
                    rhs=w1_sb[dc][:, ts(w, 512)],
                    start=(dc == 0),
                    stop=(dc == NCH - 1),
                )
        ctx_sb = fin.tile([NB, D], FP32, tag="ctx")
        nc.vector.tensor_tensor(ctx_sb, psf, b1r_sb, Alu.add)
        nc.scalar.dma_start(out[:], ctx_sb)

    nc.finalize()
    return nc


def kernel(h_enc, h_dec, W1, b1, W2, b2, V, bv):
    from concourse.bass_utils import run_bass_kernel_spmd

    h_enc = np.asarray(h_enc, dtype=np.float32)
    h_dec = np.asarray(h_dec, dtype=np.float32)
    W1 = np.asarray(W1, dtype=np.float32)
    b1 = np.asarray(b1, dtype=np.float32)
    W2 = np.asarray(W2, dtype=np.float32)
    b2 = np.asarray(b2, dtype=np.float32)
    V = np.asarray(V, dtype=np.float32)

    if "nc" not in _cache:
        _cache["nc"] = _build()
    nc = _cache["nc"]

    f16 = np.float16
    w1t = np.ascontiguousarray(W1.T).reshape(NCH, P, D).astype(f16)
    vt = V.reshape(NCH, P).T  # [P, NCH]
    vrep = np.ascontiguousarray(np.broadcast_to(vt[:, :, None], (P, NCH, P))).astype(f16)
    b1r = np.ascontiguousarray(np.broadcast_to(b1[None, :], (NB, D))).astype(np.float32)
    dec_all = (h_dec @ W2.T + b1 + b2).astype(np.float32)  # [B, D]

    h16 = h_enc.astype(f16)  # [B, L, D]

    in_maps = []
    for core in range(NCORES):
        sl = slice(core * NB, (core + 1) * NB)
        # hq[b, q, c, l] = h16[b, l, c*128+q]
        hq = np.ascontiguousarray(
            h16[sl].reshape(NB, L, NCH, P).transpose(0, 3, 2, 1)
        )
        biasd = np.ascontiguousarray(
            dec_all[sl].T.reshape(NCH, P, NB).transpose(1, 0, 2)
        )
        in_maps.append(
            {
                "hq": hq,
                "w1t": w1t,
                "vrep": vrep,
                "biasd": biasd,
                "b1r": b1r,
            }
        )

    res = run_bass_kernel_spmd(nc, in_maps, core_ids=list(range(NCORES)))
    globals()["LAST_RES"] = res
    outs = [res.results[core]["ctx_out"] for core in range(NCORES)]
    return np.concatenate(outs, axis=0).astype(np.float32)
